# revision 62
# baseline (speedup 1.0000x reference)
"""Trainium2 Bass kernel: NKQuantizer2 top-k masking (k=8).

reference:  kh = topk_hot(x, 8)          # [B,S,Q] 0/1 mask, top-8 per token
            out = einsum('bsq,eq->bse', kh, W)

Per token: out[t] = sum_{q in top8(x[t])} W[:, q] -- an 8-way embedding
gather-sum from W.T [Q, E].

Strategy (data-parallel over tokens across 8 cores; WT bf16 in HBM).
Per 128-token tile on each core:
  1. HWDGE DMA x tile [128, 8192] f32 HBM->SBUF (SP ring, 3-deep)
  2. DVE Max8 -> top-8 values; DVE MaxIndex -> their indices (exact,
     duplicate values resolved to successive positions, matching
     jax.lax.top_k first-occurrence tie-break)
  3. ONE SWDGE multi-index gather: G[p, j, :] = WT[idx8[p, j], :]
     (1024 descriptors of 1 KiB in a single instruction -- one Q7
     descriptor-gen pass, one DVE semaphore wait)
  4. GPSIMD tensor_tensor add tree over the j axis (3 levels, Pool
     engine -- DVE stays free): g8 = sum_j G[:, j, :]
  5. SWDGE store g8 -> out rows

Every DMA instruction may carry at most ONE semaphore wait (walrus
limit). Deps whose source is a Pool-engine instruction are demoted to
nosync ordering edges for SWDGE DMAs: their descriptor generation runs
on the Pool sequencer AFTER that instruction completed (program order),
so the semaphore would be redundant.
"""

import numpy as np
import ml_dtypes

import concourse.bass as bass
import concourse.mybir as mybir
import concourse.tile as tile
from concourse.bass_utils import run_bass_kernel_spmd
from concourse.tile_rust import add_dep_helper

B, S, Q, E, TOPK = 4, 2048, 8192, 512, 8
N_CORES = 8
P = 128
T_TOTAL = B * S                 # 8192 tokens
T_CORE = T_TOTAL // N_CORES     # 1024 tokens per core

F32 = mybir.dt.float32
BF16 = mybir.dt.bfloat16
U32 = mybir.dt.uint32


def build_bass(t_core=T_CORE, q=Q, e=E):
    """Build the per-core Bass program (SPMD: same program on all cores)."""
    n_tiles = t_core // P
    xbufs = min(3, n_tiles)

    nc = bass.Bass(trn_type="TRN2", target_bir_lowering=False)
    x_d = nc.dram_tensor("x", [t_core, q], F32, kind="ExternalInput")
    wt_d = nc.dram_tensor("wt", [q, e], BF16, kind="ExternalInput")
    out_d = nc.dram_tensor("out", [t_core, e], F32, kind="ExternalOutput")

    pool_insts = {}  # name -> Pool-engine compute instruction
    tt3_names = set()

    def demote_pool_deps(inst, keep=()):
        """Demote an instruction's sync deps on Pool COMPUTE instructions to
        nosync edges: the Pool engine executes serially, so anything issued
        later on Pool (compute or SWDGE descriptor-gen) starts only after
        those completed. Deps on DMA instructions (async data landing) are
        kept."""
        keep_names = {k.ins.name for k in keep}
        for name in list(inst.ins.sync_dependency_names()):
            if name in pool_insts and name not in keep_names:
                inst.ins.try_remove_dependency(name)
                add_dep_helper(inst.ins, pool_insts[name].ins, False, "Pool order")
        return inst

    with tile.TileContext(nc) as tc:
        with (
            tc.tile_pool(name="xpool", bufs=xbufs) as xpool,
            tc.tile_pool(name="spool", bufs=n_tiles) as spool,
            tc.tile_pool(name="ipool", bufs=n_tiles) as ipool,
            tc.tile_pool(name="Gpool", bufs=n_tiles) as Gpool,
            tc.tile_pool(name="gpool", bufs=n_tiles) as gpool,
        ):
            xts = [xpool.tile([P, q], F32, name="xt", tag="xt") for _ in range(xbufs)]
            i_idxs = []
            xls = []
            xl_names = set()
            swdge_dmas = []
            swdge_names = {}
            pending = []
            a_pending = []
            finals = {}
            dve_insts = {}
            dve_latest = [None]

            def reg_dve(op):
                dve_insts[op.ins.name] = op
                dve_latest[0] = op
                return op

            def emit_reduce_store(i, Gq, b_g):
                # Two in-place quarter adds on DVE (ordered after the newest
                # max_index via a nosync edge so they never block top-k),
                # final add on Pool, then the SWDGE store.
                # A nop ladder waits each quarter's own completion sem (one
                # wait per instruction), then one merged add folds quarters
                # 2,3 into 0,1.
                # The completion-wait ladder rides the POOL sequencer (it
                # has slack; DVE is the bottleneck). Pool nop k waits gather
                # k's own completion sem; the DVE add then needs only ONE
                # cross-engine wait on the last nop's Pool tick, which by
                # Pool program order implies the whole ladder passed.
                ladder = []
                for bg in b_g:
                    vn = nc.gpsimd.nop()
                    add_dep_helper(vn.ins, bg.ins, True, "slice done")
                    pool_insts[vn.ins.name] = vn
                    ladder.append(vn)
                vn1 = ladder[-1]
                va0 = reg_dve(nc.vector.tensor_tensor(
                    out=Gq[:, 0:4, :], in0=Gq[:, 0:4, :], in1=Gq[:, 4:8, :],
                    op=mybir.AluOpType.add,
                ))
                add_dep_helper(va0.ins, vn1.ins, True, "ladder passed")
                for bg in b_g:
                    va0.ins.try_remove_dependency(bg.ins.name)
                    add_dep_helper(va0.ins, bg.ins, False, "via ladder")
                va1 = reg_dve(nc.vector.tensor_tensor(
                    out=Gq[:, 0:2, :], in0=Gq[:, 0:2, :], in1=Gq[:, 2:4, :],
                    op=mybir.AluOpType.add,
                ))
                for vn in ladder:
                    # The add must sit AFTER the whole ladder in DVE program
                    # order -- without these edges the scheduler may hoist it
                    # above the waits.
                    add_dep_helper(va1.ins, vn.ins, False, "after ladder")
                    add_dep_helper(vn.ins, i_idxs[-1].ins, False, "after topk")
                for bg in b_g:
                    va1.ins.try_remove_dependency(bg.ins.name)
                    add_dep_helper(va1.ins, bg.ins, False, "via ring nop")
                # Hoist any remaining cross-engine sync deps (scheduler
                # artifacts on tail tiles) onto their own nops; demote
                # same-engine (DVE) deps to program order.
                for name in list(va1.ins.sync_dependency_names()):
                    if name in dve_insts:
                        va1.ins.try_remove_dependency(name)
                        add_dep_helper(va1.ins, dve_insts[name].ins, False, "DVE order")
                    elif name in swdge_names:
                        va1.ins.try_remove_dependency(name)
                        hn = reg_dve(nc.vector.nop())
                        add_dep_helper(hn.ins, swdge_names[name].ins, True, "hoist")
                        add_dep_helper(va1.ins, hn.ins, False, "after hoist nop")
                va2 = va1
                for v in (va0, va1):
                    # Keep the whole reduce AFTER the newest max_index in DVE
                    # program order -- it waits on the gather round trip and
                    # must never stall the top-k stream.
                    add_dep_helper(v.ins, i_idxs[-1].ins, False, "after topk")
                g8 = gpool.tile([P, e], F32, name="g8", tag="g8")
                tt3 = nc.gpsimd.tensor_tensor(
                    out=g8[:], in0=Gq[:, 0, :], in1=Gq[:, 1, :],
                    op=mybir.AluOpType.add,
                )
                pool_insts[tt3.ins.name] = tt3
                tt3_names.add(tt3.ins.name)
                st = nc.gpsimd.dma_start(
                    out_d[i * P : (i + 1) * P, :], g8[:]
                )
                demote_pool_deps(st)
                swdge_dmas.append(st)
                swdge_names[st.ins.name] = st
                finals["tt"] = tt3
                finals["va"] = va2
            for i in range(n_tiles):
                xt = xts[i % xbufs]
                # Load x in 4 column chunks: a single 4 MiB DMA queues 256
                # KiB per SDMA engine, which delays the gathers' completion
                # semaphores by up to ~9 us; 1 MiB chunks cap that at ~2.5 us.
                # All chunks ride the SP HWDGE ring in order, so chunk 3's
                # completion implies chunks 0-2 have landed on every engine.
                chunks = []
                spn = None
                if i >= xbufs:
                    # One SP nop carries every xt-slot WAR: the old slot
                    # readers are max/max_index of tile i-xbufs, covered by
                    # that tile's max_index tick. Tiles 0..xbufs-1 write
                    # fresh buffers -- any DVE dep on their chunks is a
                    # tracking artifact, dropped without replacement.
                    spn = nc.sync.nop()
                    add_dep_helper(spn.ins, i_idxs[i - xbufs].ins, True, "xt WAR")
                for c4 in range(4):
                    lo, hi = c4 * (q // 4), (c4 + 1) * (q // 4)
                    xc = nc.sync.dma_start(
                        xt[:, lo:hi], x_d[i * P : (i + 1) * P, lo:hi]
                    )
                    for name in list(xc.ins.sync_dependency_names()):
                        if name in dve_insts:
                            xc.ins.try_remove_dependency(name)
                        elif name in xl_names:
                            # old-chunk WAW, redundant under the WAR
                            xc.ins.try_remove_dependency(name)
                    if spn is not None:
                        add_dep_helper(xc.ins, spn.ins, False, "after WAR nop")
                    chunks.append(xc)
                xls.extend(chunks)
                for xc in chunks:
                    xl_names.add(xc.ins.name)

                s8 = spool.tile([P, 8], F32, name="s8", tag="s8")
                vm = reg_dve(nc.vector.max(out=s8[:], in_=xt[:]))
                # One wait: the last chunk's completion implies the rest.
                for xc in chunks[:-1]:
                    vm.ins.try_remove_dependency(xc.ins.name)
                    add_dep_helper(vm.ins, xc.ins, False, "ring order")
                idx8 = ipool.tile([P, 8], U32, name="idx8", tag="idx8")
                i_idx = reg_dve(nc.vector.max_index(
                    out=idx8[:], in_max=s8[:], in_values=xt[:]
                ))
                # max8 already waited on the last chunk; DVE order covers it.
                for xc in chunks:
                    i_idx.ins.try_remove_dependency(xc.ins.name)
                    add_dep_helper(i_idx.ins, xc.ins, False, "via max8 wait")
                i_idxs.append(i_idx)

                # 8 single-offset gathers (the HW-supported form) arranged as
                # 4 CCE-accumulate chains of depth 2 into quarters:
                #   Gq[:, j, :]  = WT[idx8[:, j]]        (bypass, bf16->f32)
                #   Gq[:, j, :] += WT[idx8[:, j+4]]      (CCE add)
                # Chain waits are hoisted onto Pool nops so each DMA carries
                # at most one (sem-lane) wait.
                Gq = Gpool.tile([P, TOPK, e], BF16, name="Gq", tag="Gq")
                if i > 0:
                    # Hoist the idx8-ready (DVE) wait for this tile's gathers.
                    n = nc.gpsimd.nop()
                    add_dep_helper(n.ins, i_idx.ins, True, "idx8 ready")
                    pool_insts[n.ins.name] = n

                def gather(j, acc, _Gq=Gq, _idx8=idx8):
                    gd = nc.gpsimd.indirect_dma_start(
                        out=_Gq[:, j, :],
                        out_offset=None,
                        in_=wt_d[:],
                        in_offset=bass.IndirectOffsetOnAxis(
                            ap=_idx8[:, j : j + 1], axis=0
                        ),
                        compute_op=mybir.AluOpType.bypass,
                    )
                    demote_pool_deps(gd)
                    swdge_dmas.append(gd)
                    swdge_names[gd.ins.name] = gd
                    return gd

                a_g = [gather(j, False) for j in range(TOPK)]
                if i > 0:
                    # The DVE wait rides the tile-entry nop; drop it from the
                    # gathers (desc-gen follows the nop in program order).
                    for gd in a_g:
                        gd.ins.try_remove_dependency(i_idx.ins.name)
                        add_dep_helper(gd.ins, n.ins, False, "after idx8 nop")

                # Defer this tile's reduce+store until after the NEXT tile's
                # max/max_index so the gather round trip never stalls top-k.
                pending.append((i, Gq, a_g))
                if i > 1:
                    emit_reduce_store(*pending.pop(0))
            # drain the deferred final tiles
            while pending:
                emit_reduce_store(*pending.pop(0))

            # Quiesce outstanding proc ticks with single-wait SP nops so the
            # kernel-tail drain finds its required ticks already observed
            # (the drain itself may carry only one sync wait).
            tail = xls + swdge_dmas[-9:] + [finals["tt"], finals["va"]] + i_idxs[-1:]
            for tgt in tail:
                n = nc.sync.nop()
                add_dep_helper(n.ins, tgt.ins, True, "tail quiesce")

    # Post-pass: walrus allows ONE sync wait per instruction. Drop waits
    # that are provably redundant: (a) a wait on the instruction's own
    # engine sem (program order), (b) an earlier SWDGE/DMASW lane tick when
    # a later lane tick is also waited (lanes are assigned round-robin in
    # ring order and each SDMA engine drains the ring FIFO, so the later
    # DMA's completion implies the earlier one's data landed).
    for f in nc.m.functions:
        for b in f.blocks:
            for ins in b.instructions:
                si = ins.sync_info
                if not si or len(si.on_wait) <= 1:
                    continue
                eng = str(getattr(ins, "engine", "")).split(".")[-1]
                own = f"{eng}_"
                keep = [w for w in si.on_wait if not w.ant_name.startswith(own)]
                if ins.name in tt3_names:
                    dve = [w for w in keep if w.ant_name.startswith("DVE_")]
                    if dve:
                        keep = dve
                sw = [w for w in keep if w.ant_name.startswith("DMASW")]
                if len(sw) > 1:
                    # ring ordinal: DMA n -> lane n%8, count (n//8+1)*16
                    def ordinal(w):
                        lane = int(w.ant_name[5:].split("_")[0])
                        return (w.wait_value // 16 - 1) * 8 + lane
                    best = max(sw, key=ordinal)
                    keep = [w for w in keep if not (w.ant_name.startswith("DMASW") and w is not best)]
                if len(keep) >= 1 and len(keep) < len(si.on_wait):
                    ins.sync_info = mybir.SyncInfo(
                        on_wait=keep, on_update=list(si.on_update)
                    )
    return nc


def _prep_wt(W: np.ndarray) -> np.ndarray:
    """W [e, q] f32 -> WT [q, e] bf16 contiguous."""
    return np.ascontiguousarray(W.T).astype(ml_dtypes.bfloat16)


_CACHED = {}


def _get_nc():
    if "nc" not in _CACHED:
        _CACHED["nc"] = build_bass()
    return _CACHED["nc"]


def kernel(x: np.ndarray, W: np.ndarray) -> np.ndarray:
    x = np.asarray(x, dtype=np.float32)
    W = np.asarray(W, dtype=np.float32)
    assert x.shape == (B, S, Q) and W.shape == (E, Q)

    nc = _get_nc()
    xf = x.reshape(T_TOTAL, Q)
    WT = _prep_wt(W)
    in_maps = [
        {
            "x": np.ascontiguousarray(xf[c * T_CORE : (c + 1) * T_CORE]),
            "wt": WT,
        }
        for c in range(N_CORES)
    ]
    res = run_bass_kernel_spmd(nc, in_maps, core_ids=list(range(N_CORES)))
    out = np.concatenate([r["out"] for r in res.results], axis=0)
    return np.ascontiguousarray(out.reshape(B, S, E).astype(np.float32))
